# revision 18
# baseline (speedup 1.0000x reference)
"""PhasorLayer TRN2 kernel: data-parallel over batch across 8 NeuronCores.

Math (per batch row m):
  u     = x @ [Wk|Wq|wsum]^T + [bk|bq|sum_bv]          (KQS gemm, N=129)
  align = 64 - 2*sum_p sin^2((pi/2)*(tanh(uk)-tanh(uq)))
  gain  = softplus(align/64 + 0.5);  s = align*gain/64
  w     = x @ Wv^T + bv        (V including bias)
  muw   = mean(w);  varw = mean(w^2) - muw^2
  inv   = rsqrt(s^2*varw + 1e-5);  a = s*inv;  c = a*muw
  out   = x + a*(w @ Wo'^T) - c*w1 + r
  where Wo' = Wo * ln_g (cols), w1 = rowsum(Wo'), r = ln_b @ Wo^T + bo
"""

import sys

sys.path.insert(0, "/opt/trn_rl_repo")

import math
import os
from contextlib import ExitStack

import numpy as np

import concourse.bass as bass
import concourse.mybir as mybir
import concourse.tile as tile
from concourse.alu_op_type import AluOpType
from concourse.bass_utils import run_bass_kernel_spmd
from concourse.mybir import dt
from concourse.tile_cfg import (
    BassTileBranchHintPlaceholder,
    BassTileConditionalBlock,
    BassTileCriticalSection,
    BassTileLoopBlock,
    BassTileSwitchBlock,
    TileBranchInst,
)
from concourse.vector_clock import ScopedClock

B, D, P = 8192, 4096, 64
NCORES = 8
M = B // NCORES  # 1024 batch rows per core
MT = M // 128    # 8 m-tiles
KD = D // 128    # 32 dim tiles
NB = D // 512    # 8 n-blocks
PI = math.pi
EPS = 1e-5
F32 = dt.float32
AF = mybir.ActivationFunctionType

USE_F32R = True  # big GEMMs in float32r (4x PE throughput); KQS stays fp32
MMDT = dt.float32r if USE_F32R else dt.float32

_SKIP_SPLIT = (
    BassTileBranchHintPlaceholder,
    BassTileConditionalBlock,
    BassTileCriticalSection,
    BassTileLoopBlock,
    BassTileSwitchBlock,
    TileBranchInst,
)


class LegalTileContext(tile.TileContext):
    """TileContext legalized to <=1 semaphore wait per instruction.

    This container's walrus rejects instructions with >1 sync wait. Extra
    waits are peeled onto single-wait NoOps on the same engine.
    """

    def _lower_ordered_insts(self, ordered):
        for insts in ordered.values():
            out = []
            for inst in insts:
                si = getattr(inst, "sync_info", None)
                if (
                    si is not None
                    and len(si.on_wait) > 1
                    and not isinstance(inst, _SKIP_SPLIT)
                ):
                    waits = list(si.on_wait)
                    for w in waits[:-1]:
                        nop = mybir.InstNoOp(
                            name=self.nc.get_next_instruction_name(),
                            text_hint="wait_split",
                            bass_nofuse=True,
                            engine=inst.engine,
                            sync_info=mybir.SyncInfo(on_wait=[w], on_update=[]),
                        )
                        out.append(nop)
                    inst.sync_info = mybir.SyncInfo(
                        on_wait=[waits[-1]], on_update=list(si.on_update)
                    )
                out.append(inst)
            insts[:] = out
        super()._lower_ordered_insts(ordered)

    def _drain_and_barrier(self, tick_clock, wait_clock):
        drain_inst = self.nc.sync.drain()
        wait_clock.add_sem_waits(
            drain_inst.ins, ScopedClock({None: tick_clock.global_clock})
        )
        si = drain_inst.ins.sync_info
        if si is not None and len(si.on_wait) > 1:
            waits = list(si.on_wait)
            drain_inst.ins.sync_info = mybir.SyncInfo(
                on_wait=[waits[0]], on_update=list(si.on_update)
            )
            for w in waits[1:]:
                nop = self.nc.sync.nop(nofuse=True, hint="wait_split")
                nop.ins.sync_info = mybir.SyncInfo(on_wait=[w], on_update=[])
        self.nc.all_engine_barrier()
        assert self.sems is not None
        popped = self.nc._tile_sem_poison_stack.pop()
        assert popped is self._sem_poison
        self.nc.clear_and_free_semaphores(list(self.sems.allocated().values()))
        self.nc.all_engine_barrier()


def _f32(ap):
    # fp32r-typed tiles carry full fp32 bytes; bitcast back for fp32 matmuls
    return ap.bitcast(dt.float32) if USE_F32R else ap


def _r(ap):
    return ap.bitcast(dt.float32r) if USE_F32R else ap


def build_nc():
    nc = bass.Bass()
    x_d = nc.declare_dram_parameter("x", [M, D], F32, isOutput=False)
    xt_d = nc.declare_dram_parameter("xt", [D, M], MMDT, isOutput=False)
    wvt_d = nc.declare_dram_parameter("wvt", [D, D], MMDT, isOutput=False)
    wo2t_d = nc.declare_dram_parameter("wo2t", [D, D], MMDT, isOutput=False)
    wkqs_d = nc.declare_dram_parameter("wkqs", [D, 129], F32, isOutput=False)
    brow_d = nc.declare_dram_parameter("brow", [128, 129], F32, isOutput=False)
    bvr_d = nc.declare_dram_parameter("bvr", [128, KD], F32, isOutput=False)
    w1m_d = nc.declare_dram_parameter("w1m", [128, D], F32, isOutput=False)
    rm_d = nc.declare_dram_parameter("rm", [128, D], F32, isOutput=False)
    out_d = nc.declare_dram_parameter("out", [M, D], F32, isOutput=True)

    wt_dram = nc.dram_tensor("wt_scr", [KD, 128, M], MMDT)
    ssq_dram = nc.dram_tensor("ssq_scr", [1, M], F32)

    with ExitStack() as ctx:
        tc = ctx.enter_context(LegalTileContext(nc))
        sb_small = ctx.enter_context(tc.tile_pool(name="small", bufs=1))

        ones_t = sb_small.tile((128, 1), F32, name="ones", tag="ones")
        nc.vector.memset(ones_t[:], 1.0)
        half_t = sb_small.tile((128, 1), F32, name="half", tag="half")
        nc.vector.memset(half_t[:], 0.5)
        eps_t = sb_small.tile((128, 1), F32, name="epsb", tag="epsb")
        nc.vector.memset(eps_t[:], EPS)
        brow_t = sb_small.tile((128, 129), F32, name="browt", tag="browt")
        nc.sync.dma_start(brow_t[:], brow_d[:, :])
        bvr_t = sb_small.tile((128, KD), F32, name="bvrt", tag="bvrt")
        nc.sync.dma_start(bvr_t[:], bvr_d[:, :])

        def col_tile(nm):
            return sb_small.tile((128, MT), F32, name=nm, tag=nm)

        red_all = col_tile("red_all")
        align_all = col_tile("align_all")
        e1_all = col_tile("e1_all")
        gain_all = col_tile("gain_all")
        s2_all = col_tile("s2_all")
        mu_all = col_tile("mu_all")
        ssq_all = col_tile("ssq_all")
        musq_all = col_tile("musq_all")
        var_all = col_tile("var_all")
        s_all = col_tile("s_all")
        s_sq_all = col_tile("s_sq_all")
        q_all = col_tile("q_all")
        q2_all = col_tile("q2_all")
        inv_all = col_tile("inv_all")
        a_all = col_tile("a_all")
        c_all = col_tile("c_all")
        cneg_all = col_tile("cneg_all")
        acc_sb = sb_small.tile((1, M), F32, name="acc_sb", tag="acc_sb")

        # ---------------- phase 1: KQS + GEMM1 (xt resident) ----------------
        with ExitStack() as p1:
            sb_xt = p1.enter_context(tc.tile_pool(name="xtp", bufs=1))
            sb_s1 = p1.enter_context(tc.tile_pool(name="s1", bufs=2))
            ps_kq = p1.enter_context(tc.tile_pool(name="pskq", bufs=2, space="PSUM"))
            ps_v = p1.enter_context(tc.tile_pool(name="psv", bufs=2, space="PSUM"))
            ps_acc = p1.enter_context(tc.tile_pool(name="psacc", bufs=1, space="PSUM"))

            xt_ts = []
            for j in range(KD):
                t = sb_xt.tile((128, M), MMDT, name=f"xt{j}", tag=f"xt{j}")
                nc.sync.dma_start(t[:], xt_d[j * 128 : (j + 1) * 128, :])
                xt_ts.append(t)
            wkq_ts = []
            for j in range(KD):
                t = sb_xt.tile((128, 129), F32, name=f"wkq{j}", tag=f"wkq{j}")
                nc.sync.dma_start(t[:], wkqs_d[j * 128 : (j + 1) * 128, :])
                wkq_ts.append(t)

            # KQS gemm + per-row scalar pipeline (fp32 matmul for accuracy)
            for t in range(MT):
                kq_ps = ps_kq.tile((128, 129), F32, name="kq_ps", tag="kq")
                for j in range(KD):
                    nc.tensor.matmul(
                        kq_ps[:],
                        _f32(xt_ts[j][:, t * 128 : (t + 1) * 128]),
                        wkq_ts[j][:],
                        start=(j == 0),
                        stop=(j == KD - 1),
                    )
                u_t = sb_s1.tile((128, 129), F32, name="u_t", tag="u")
                nc.vector.tensor_add(u_t[:], kq_ps[:], brow_t[:])
                th_t = sb_s1.tile((128, 128), F32, name="th_t", tag="th")
                nc.scalar.activation(th_t[:], u_t[:, 0:128], AF.Tanh)
                d_t = sb_s1.tile((128, 64), F32, name="d_t", tag="d")
                nc.vector.tensor_sub(d_t[:], th_t[:, 0:64], th_t[:, 64:128])
                sn_t = sb_s1.tile((128, 64), F32, name="sn_t", tag="sn")
                nc.scalar.activation(sn_t[:], d_t[:], AF.Sin, scale=PI / 2)
                sq_t = sb_s1.tile((128, 64), F32, name="sq_t", tag="snsq")
                nc.scalar.activation(
                    sq_t[:], sn_t[:], AF.Square, accum_out=red_all[:, t : t + 1]
                )
                nc.vector.tensor_scalar(
                    align_all[:, t : t + 1],
                    red_all[:, t : t + 1],
                    -2.0,
                    float(P),
                    AluOpType.mult,
                    AluOpType.add,
                )
                nc.scalar.activation(
                    e1_all[:, t : t + 1],
                    align_all[:, t : t + 1],
                    AF.Exp,
                    bias=half_t[:],
                    scale=1.0 / P,
                )
                nc.scalar.activation(
                    gain_all[:, t : t + 1], e1_all[:, t : t + 1], AF.Ln, bias=1.0
                )
                nc.vector.tensor_mul(
                    s2_all[:, t : t + 1],
                    align_all[:, t : t + 1],
                    gain_all[:, t : t + 1],
                )
                nc.scalar.activation(
                    mu_all[:, t : t + 1], u_t[:, 128:129], AF.Copy, scale=1.0 / D
                )

            # GEMM1: w^T = Wv @ x^T + bv, plus sum(w^2) partition-reduce
            acc_ps0 = ps_acc.tile((1, 512), F32, name="acc_ps0", tag="acc0")
            acc_ps1 = ps_acc.tile((1, 512), F32, name="acc_ps1", tag="acc1")
            for kd in range(KD):
                v_ps0 = ps_v.tile((128, 512), F32, name="v_ps0", tag="v0")
                v_ps1 = ps_v.tile((128, 512), F32, name="v_ps1", tag="v1")
                for j in range(KD):
                    wv_t = sb_s1.tile((128, 128), MMDT, name="wv_t", tag="wv", bufs=3)
                    nc.sync.dma_start(
                        wv_t[:],
                        wvt_d[j * 128 : (j + 1) * 128, kd * 128 : (kd + 1) * 128],
                    )
                    nc.tensor.matmul(
                        v_ps0[:],
                        wv_t[:],
                        xt_ts[j][:, 0:512],
                        start=(j == 0),
                        stop=(j == KD - 1),
                    )
                    nc.tensor.matmul(
                        v_ps1[:],
                        wv_t[:],
                        xt_ts[j][:, 512:1024],
                        start=(j == 0),
                        stop=(j == KD - 1),
                    )
                wt_t = sb_s1.tile((128, M), F32, name="wt_t", tag="wt")
                nc.vector.tensor_scalar(
                    wt_t[:, 0:512], v_ps0[:], bvr_t[:, kd : kd + 1], None, AluOpType.add
                )
                nc.vector.tensor_scalar(
                    wt_t[:, 512:1024],
                    v_ps1[:],
                    bvr_t[:, kd : kd + 1],
                    None,
                    AluOpType.add,
                )
                sqw_t = sb_s1.tile((128, M), F32, name="sqw_t", tag="sqw")
                nc.scalar.activation(sqw_t[:], wt_t[:], AF.Square)
                nc.tensor.matmul(
                    acc_ps0[:],
                    ones_t[:],
                    sqw_t[:, 0:512],
                    start=(kd == 0),
                    stop=(kd == KD - 1),
                )
                nc.tensor.matmul(
                    acc_ps1[:],
                    ones_t[:],
                    sqw_t[:, 512:1024],
                    start=(kd == 0),
                    stop=(kd == KD - 1),
                )
                nc.sync.dma_start(wt_dram[kd, :, :], _r(wt_t[:]))

            # ssq bounce: [1, M] -> DRAM -> [128, MT] columns
            nc.scalar.copy(acc_sb[:, 0:512], acc_ps0[:])
            nc.scalar.copy(acc_sb[:, 512:1024], acc_ps1[:])
            nc.sync.dma_start(ssq_dram[:, :], acc_sb[:])
            for t in range(MT):
                nc.sync.dma_start(
                    ssq_all[:, t : t + 1],
                    ssq_dram[0:1, t * 128 : (t + 1) * 128].transpose([1, 0]),
                )

        # ---------------- scalar finalize ----------------
        nc.scalar.activation(musq_all[:], mu_all[:], AF.Square)
        nc.vector.tensor_scalar(
            var_all[:], ssq_all[:], 1.0 / D, None, AluOpType.mult
        )
        nc.vector.tensor_sub(var_all[:], var_all[:], musq_all[:])
        nc.scalar.activation(s_all[:], s2_all[:], AF.Copy, scale=1.0 / P)
        nc.scalar.activation(s_sq_all[:], s_all[:], AF.Square)
        nc.vector.tensor_mul(q_all[:], var_all[:], s_sq_all[:])
        nc.scalar.activation(q2_all[:], q_all[:], AF.Sqrt, bias=eps_t[:])
        nc.vector.reciprocal(inv_all[:], q2_all[:])
        nc.vector.tensor_mul(a_all[:], s_all[:], inv_all[:])
        nc.vector.tensor_mul(c_all[:], a_all[:], mu_all[:])
        nc.vector.tensor_scalar(
            cneg_all[:], c_all[:], -1.0, None, AluOpType.mult
        )

        # ---------------- phase 2: GEMM2 + epilogue (wt resident) ----------------
        with ExitStack() as p2:
            sb_wt = p2.enter_context(tc.tile_pool(name="wtp", bufs=1))
            sb_s2 = p2.enter_context(tc.tile_pool(name="s2", bufs=2))
            ps_p = p2.enter_context(tc.tile_pool(name="psp", bufs=1, space="PSUM"))

            wt_res = []
            for k in range(KD):
                t = sb_wt.tile((128, M), MMDT, name=f"wtr{k}", tag=f"wtr{k}")
                nc.sync.dma_start(t[:], wt_dram[k, :, :])
                wt_res.append(t)
            w1_res = sb_wt.tile((128, D), F32, name="w1_res", tag="w1_res")
            nc.sync.dma_start(w1_res[:], w1m_d[:, :])
            r_res = sb_wt.tile((128, D), F32, name="r_res", tag="r_res")
            nc.sync.dma_start(r_res[:], rm_d[:, :])

            for nb in range(NB):
                nsl = slice(nb * 512, (nb + 1) * 512)
                p_tiles = [
                    ps_p.tile((128, 512), F32, name=f"pp{mt}", tag=f"pp{mt}")
                    for mt in range(MT)
                ]
                for k in range(KD):
                    wo_t = sb_s2.tile((128, 512), MMDT, name="wo_t", tag="wo")
                    nc.sync.dma_start(wo_t[:], wo2t_d[k * 128 : (k + 1) * 128, nsl])
                    for mt in range(MT):
                        nc.tensor.matmul(
                            p_tiles[mt][:],
                            wt_res[k][:, mt * 128 : (mt + 1) * 128],
                            wo_t[:],
                            start=(k == 0),
                            stop=(k == KD - 1),
                        )
                for mt in range(MT):
                    msl = slice(mt * 128, (mt + 1) * 128)
                    xe_t = sb_s2.tile((128, 512), F32, name="xe_t", tag="xe")
                    nc.sync.dma_start(xe_t[:], x_d[msl, nsl])
                    t1_t = sb_s2.tile((128, 512), F32, name="t1_t", tag="t1")
                    nc.vector.scalar_tensor_tensor(
                        t1_t[:],
                        p_tiles[mt][:],
                        a_all[:, mt : mt + 1],
                        xe_t[:],
                        AluOpType.mult,
                        AluOpType.add,
                    )
                    u2_t = sb_s2.tile((128, 512), F32, name="u2_t", tag="u2")
                    nc.vector.scalar_tensor_tensor(
                        u2_t[:],
                        w1_res[:, nsl],
                        cneg_all[:, mt : mt + 1],
                        r_res[:, nsl],
                        AluOpType.mult,
                        AluOpType.add,
                    )
                    oe_t = sb_s2.tile((128, 512), F32, name="oe_t", tag="oe")
                    nc.vector.tensor_add(oe_t[:], t1_t[:], u2_t[:])
                    nc.sync.dma_start(out_d[msl, nsl], oe_t[:])
    return nc


def kernel(**inputs):
    x = np.asarray(inputs["x"], dtype=np.float32)
    Wk = np.asarray(inputs["Wk"], dtype=np.float32)
    bk = np.asarray(inputs["bk"], dtype=np.float32)
    Wq = np.asarray(inputs["Wq"], dtype=np.float32)
    bq = np.asarray(inputs["bq"], dtype=np.float32)
    Wv = np.asarray(inputs["Wv"], dtype=np.float32)
    bv = np.asarray(inputs["bv"], dtype=np.float32)
    ln_g = np.asarray(inputs["ln_g"], dtype=np.float32)
    ln_b = np.asarray(inputs["ln_b"], dtype=np.float32)
    Wo = np.asarray(inputs["Wo"], dtype=np.float32)
    bo = np.asarray(inputs["bo"], dtype=np.float32)

    Wo2T = np.ascontiguousarray((Wo * ln_g[None, :]).T)  # [k, n] = Wo'[n, k]
    w1 = Wo2T.sum(axis=0)  # [n]
    r = ln_b @ Wo.T + bo  # [n]
    WvT = np.ascontiguousarray(Wv.T)  # [j, k]
    wsum = Wv.sum(axis=0)  # [j]
    wkqs = np.ascontiguousarray(
        np.concatenate([Wk.T, Wq.T, wsum[:, None]], axis=1), dtype=np.float32
    )  # [D, 129]
    brow = np.concatenate([bk, bq, [bv.sum()]]).astype(np.float32)  # [129]
    brow_mat = np.ascontiguousarray(np.broadcast_to(brow, (128, 129)))
    w1_mat = np.ascontiguousarray(np.broadcast_to(w1, (128, D)), dtype=np.float32)
    r_mat = np.ascontiguousarray(np.broadcast_to(r, (128, D)), dtype=np.float32)
    bvr = np.ascontiguousarray(bv.reshape(KD, 128).T)  # [128, KD]

    nc = build_nc()
    in_maps = []
    for c in range(NCORES):
        xc = np.ascontiguousarray(x[c * M : (c + 1) * M])
        xtc = np.ascontiguousarray(xc.T)
        in_maps.append(
            {
                "x": xc,
                "xt": xtc,
                "wvt": WvT,
                "wo2t": Wo2T,
                "wkqs": wkqs,
                "brow": brow_mat,
                "bvr": bvr,
                "w1m": w1_mat,
                "rm": r_mat,
            }
        )
    global LAST_BUILD
    LAST_BUILD = (nc, in_maps)
    trace = os.environ.get("PHASOR_TRACE") == "1"
    res = run_bass_kernel_spmd(
        nc, in_maps, core_ids=list(range(NCORES)), trace=trace
    )
    global LAST_EXEC_NS
    LAST_EXEC_NS = getattr(res, "exec_time_ns", None)
    out = np.concatenate([res.results[c]["out"] for c in range(NCORES)], axis=0)
    return out.astype(np.float32)


LAST_EXEC_NS = None
LAST_BUILD = None


if __name__ == "__main__":
    rng = np.random.default_rng(0)
    ins = {
        "x": rng.standard_normal((B, D), dtype=np.float32),
        "Wk": rng.standard_normal((P, D), dtype=np.float32) / math.sqrt(D),
        "bk": np.zeros(P, np.float32),
        "Wq": rng.standard_normal((P, D), dtype=np.float32) / math.sqrt(D),
        "bq": np.zeros(P, np.float32),
        "Wv": rng.standard_normal((D, D), dtype=np.float32) / math.sqrt(D),
        "bv": np.zeros(D, np.float32),
        "ln_g": np.ones(D, np.float32),
        "ln_b": np.zeros(D, np.float32),
        "Wo": rng.standard_normal((D, D), dtype=np.float32) / math.sqrt(D),
        "bo": np.zeros(D, np.float32),
    }
    out = kernel(**ins)
    print("out", out.shape, out.dtype, float(np.abs(out).mean()))
